# revision 1
# baseline (speedup 1.0000x reference)
"""Trainium2 Bass kernel for nn_DecSwitchedDeconv (switched per-sample double deconv).

Strategy (data-parallel over 8 cores, 32 samples/core, processed in pairs):
  - x padded to 34x34 in SBUF; stride-1 ConvTranspose == 3x3 conv with flipped
    kernel, realized as shift-offset bf16 matmuls accumulating f32 in PSUM
    (3 chunks of N=362, each within one PSUM bank).
  - Two samples per matmul via block-diagonal stationary weights
    (conv1 lhsT [128 = 2*64 cin, 64 = 2*32 cout]); conv2 additionally stacks
    dy=0/1 taps into K=128 using a row-shifted duplicate of h (6 matmuls per
    chunk instead of 9), with dy=2 folded into the upper partition half.
  - Per-sample branch weights gathered on-device with register-dynamic DMAs
    (y_index -> SP/Pool registers -> bass.ds row offsets) from pre-zero-padded
    A/B-position DRAM tables, so gathered pair tiles are block-diagonal with
    one contiguous descriptor per row.
  - bias+relu on ScalarE from PSUM; epilogue fused as (psum + b2) * z on
    VectorE, then residual add; I/O DMAs on sync (HW DGE), conv2 weight
    gathers on the gpsimd queue.
"""

import numpy as np

import concourse.bacc as bacc
import concourse.bass as bass
import concourse.mybir as mybir
import concourse.tile as tile
from concourse.bass_utils import run_bass_kernel_spmd

B, C, CSM, NB, HW = 256, 64, 32, 8, 32
M = 8                  # cores
BS = B // M            # 32 samples per core
NPAIR = BS // 2        # 16
WP = HW + 2            # 34 padded width
L = WP * WP            # 1156
NVAL = (HW - 1) * WP + HW   # 1086: contiguous span covering all valid outputs
BASE = WP + 1          # 35: flat offset of (y=1,x=1)
NCH = 3
CH = NVAL // NCH       # 362 matmul chunk (>=256 keeps float32r at 1 cyc/row)
NBUF = 4               # ping-pong depth for persistent per-pair buffers

f32 = mybir.dt.float32
bf16 = mybir.dt.bfloat16
i32 = mybir.dt.int32


def _build_bass():
    nc = bacc.Bacc(target_bir_lowering=False, debug=False)
    xs = nc.dram_tensor("xs", [BS * C, HW * HW], f32, kind="ExternalInput")
    y32 = nc.dram_tensor("y32", [BS, 1], i32, kind="ExternalInput")
    zs = nc.dram_tensor("zs", [BS * C, 1], f32, kind="ExternalInput")
    w1ga = nc.dram_tensor("w1ga", [NB * C, 9 * 2 * CSM], bf16, kind="ExternalInput")
    w1gb = nc.dram_tensor("w1gb", [NB * C, 9 * 2 * CSM], bf16, kind="ExternalInput")
    w2gla = nc.dram_tensor("w2gla", [NB * CSM, 6 * 2 * C], bf16, kind="ExternalInput")
    w2glb = nc.dram_tensor("w2glb", [NB * CSM, 6 * 2 * C], bf16, kind="ExternalInput")
    w2gua = nc.dram_tensor("w2gua", [NB * CSM, 6 * 2 * C], bf16, kind="ExternalInput")
    w2gub = nc.dram_tensor("w2gub", [NB * CSM, 6 * 2 * C], bf16, kind="ExternalInput")
    b1g = nc.dram_tensor("b1g", [NB * CSM, 1], f32, kind="ExternalInput")
    b2g = nc.dram_tensor("b2g", [NB * C, 1], f32, kind="ExternalInput")
    outd = nc.dram_tensor("out", [BS * C, HW * HW], f32, kind="ExternalOutput")

    mul = mybir.AluOpType.mult
    add = mybir.AluOpType.add

    with tile.TileContext(nc) as tc:
        # y_index as an i32 row on partition 0 — register gathers read it
        # directly (engine register loads only need partition 0)
        ybc = nc.alloc_sbuf_tensor("ybc", [1, BS], i32).ap()
        nc.sync.dma_start(ybc, bass.AP(y32.ap().tensor, 0, [[BS, 1], [1, BS]]))

        # ---- persistent ping-pong buffers (zeroed once; borders/off-blocks
        # stay zero because per-pair writes never touch them) ----
        xpads, hpads, wt1s, wt2s = [], [], [], []
        for i in range(NBUF):
            xpads.append(nc.alloc_sbuf_tensor(f"xpad{i}", [128, L], bf16).ap())
            hpads.append(nc.alloc_sbuf_tensor(f"hpad{i}", [128, L], bf16).ap())
            wt1s.append(nc.alloc_sbuf_tensor(f"wt1{i}", [128, 9 * 2 * CSM], bf16).ap())
            wt2s.append(nc.alloc_sbuf_tensor(f"wt2{i}", [128, 6 * 2 * C], bf16).ap())
            nc.vector.memset(xpads[i], 0.0)
            # hpad only needs its two contiguous border strips zeroed: the
            # relu1 span + colfix + shift-copy rewrite everything else used
            nc.vector.memset(hpads[i][0:64, 0:BASE], 0.0)
            nc.vector.memset(hpads[i][0:64, BASE + NVAL:L], 0.0)

        with (
            tc.tile_pool(name="io", bufs=3) as iop,
            tc.tile_pool(name="sml", bufs=3) as smlp,
            tc.tile_pool(name="ps1", bufs=3, space="PSUM") as ps1p,
            tc.tile_pool(name="ps2", bufs=3, space="PSUM") as ps2p,
        ):
            for p in range(NPAIR):
                bi = p % NBUF
                xpad, hpad, wt1, wt2 = xpads[bi], hpads[bi], wt1s[bi], wt2s[bi]

                xraw = iop.tile([128, HW * HW], f32, tag="xraw")
                opad = iop.tile([128, L], f32, tag="opad")
                ot = iop.tile([128, HW * HW], f32, tag="ot")
                ztile = smlp.tile([128, 1], f32, tag="z")
                b1t = smlp.tile([64, 1], f32, tag="b1")
                b2t = smlp.tile([128, 1], f32, tag="b2")

                # input loads
                nc.sync.dma_start(xraw[:, :], xs.ap()[2 * p * 64:(2 * p + 2) * 64, :])
                nc.sync.dma_start(ztile[:, :], zs.ap()[2 * p * 64:(2 * p + 2) * 64, :])

                # per-sample weight/bias gathers: register-dynamic DMA of
                # full pre-zero-padded block-diagonal rows (w1ga has sample-A
                # column positions populated, w1gb sample-B; so the gathered
                # pair tile is block-diagonal with zero blocks straight from
                # DRAM, one contiguous descriptor per partition row).
                for s in range(2):
                    r = nc.alloc_register(mybir.EngineType.SP, f"gy{p}_{s}")
                    nc.sync.load(r, ybc[0:1, 2 * p + s:2 * p + s + 1])
                    nc.sync.reg_mul(r, r, 64)
                    v64 = nc.snap(r, donate=True, min_val=0, max_val=448)
                    nc.sync.dma_start(
                        wt1[64 * s:64 * (s + 1), :],
                        (w1ga if s == 0 else w1gb).ap()[bass.ds(v64, 64), :])
                    # conv2 stacked-weight + bias gathers on the gpsimd queue
                    rp = nc.alloc_register(mybir.EngineType.Pool, f"py{p}_{s}")
                    rp2 = nc.alloc_register(mybir.EngineType.Pool, f"pz{p}_{s}")
                    nc.gpsimd.load(rp, ybc[0:1, 2 * p + s:2 * p + s + 1])
                    nc.gpsimd.reg_alu(rp2, rp, 64, mul)
                    nc.gpsimd.reg_mul(rp, rp, 32)
                    vp32 = nc.snap(rp, donate=True, min_val=0, max_val=224)
                    vp64 = nc.snap(rp2, donate=True, min_val=0, max_val=448)
                    lo, up = (w2gla, w2gua) if s == 0 else (w2glb, w2gub)
                    nc.gpsimd.dma_start(
                        wt2[32 * s:32 * (s + 1), :], lo.ap()[bass.ds(vp32, 32), :])
                    nc.gpsimd.dma_start(
                        wt2[64 + 32 * s:64 + 32 * (s + 1), :],
                        up.ap()[bass.ds(vp32, 32), :])
                    nc.gpsimd.dma_start(
                        b1t[32 * s:32 * (s + 1), :], b1g.ap()[bass.ds(vp32, 32), :])
                    nc.gpsimd.dma_start(
                        b2t[64 * s:64 * (s + 1), :], b2g.ap()[bass.ds(vp64, 64), :])

                # relu(x) into padded layout (interior only; borders stay 0)
                xpad3 = xpad.rearrange("p (h w) -> p h w", w=WP)[:, 1:HW + 1, 1:HW + 1]
                xraw3 = xraw[:, :].rearrange("p (h w) -> p h w", w=HW)
                nc.scalar.activation(xpad3, xraw3, mybir.ActivationFunctionType.Relu)

                # conv1: 3 chunks x 9 taps, then bias+relu into hpad
                for c in range(NCH):
                    ps1 = ps1p.tile([64, CH], f32)
                    for t in range(9):
                        dy, dx = divmod(t, 3)
                        off = dy * WP + dx + c * CH
                        nc.tensor.matmul(
                            ps1[:, :],
                            lhsT=wt1[:, t * 2 * CSM:(t + 1) * 2 * CSM],
                            rhs=xpad[:, off:off + CH],
                            start=(t == 0), stop=(t == 8),
                        )
                    nc.scalar.activation(
                        hpad[0:64, BASE + c * CH:BASE + (c + 1) * CH], ps1[:, :],
                        mybir.ActivationFunctionType.Relu, bias=b1t[:, :],
                    )
                # re-zero the inter-row pad columns the relu span polluted
                hp3 = hpad.rearrange("p (h w) -> p h w", w=WP)
                nc.vector.memset(hp3[0:64, 1:HW + 1, 0:WP:WP - 1], 0.0)
                # duplicate h into rows 64:128 shifted left by one image row, so
                # one matmul covers taps dy=0 (rows 0:64) and dy=1 (rows 64:128)
                nc.vector.tensor_copy(hpad[64:128, 0:L - 34], hpad[0:64, 34:L])

                # conv2 + epilogue (z * (conv + b2) fused via tensor_scalar)
                for c in range(NCH):
                    ps2 = ps2p.tile([128, CH], f32)
                    for dx in range(3):
                        nc.tensor.matmul(
                            ps2[:, :],
                            lhsT=wt2[:, dx * 2 * C:(dx + 1) * 2 * C],
                            rhs=hpad[:, dx + c * CH:dx + c * CH + CH],
                            start=(dx == 0), stop=False,
                        )
                    for dx in range(3):
                        off = WP + dx + c * CH
                        nc.tensor.matmul(
                            ps2[:, :],
                            lhsT=wt2[64:128, (3 + dx) * 2 * C:(4 + dx) * 2 * C],
                            rhs=hpad[64:128, off:off + CH],
                            start=False, stop=(dx == 2),
                        )
                    nc.vector.tensor_scalar(
                        opad[:, BASE + c * CH:BASE + (c + 1) * CH], ps2[:, :],
                        b2t[:, :], ztile[:, :], op0=add, op1=mul,
                    )

                # residual add on the valid interior, store
                opad3 = opad[:, :].rearrange("p (h w) -> p h w", w=WP)[:, 1:HW + 1, 1:HW + 1]
                ot3 = ot[:, :].rearrange("p (h w) -> p h w", w=HW)
                nc.vector.tensor_tensor(ot3, opad3, xraw3, op=add)
                nc.sync.dma_start(outd.ap()[2 * p * 64:(2 * p + 2) * 64, :], ot[:, :])

    nc.compile()
    return nc


# enable walrus's redundant-LDWEIGHTS elision (off by default in bass_utils):
# with tap-outer loops, consecutive matmuls share the stationary operand.
import concourse.bass_utils as _bu
if not getattr(_bu, "_ldw_opt_patched", False):
    _orig_run_command = _bu.run_command
    def _run_command_ldw(argv, **kw):
        argv = list(argv)  # ldw-opt=true fails walrus visitInstLdweights; keep default
        return _orig_run_command(argv, **kw)
    _bu.run_command = _run_command_ldw
    _bu._ldw_opt_patched = True

_NC = None


def _get_nc():
    global _NC
    if _NC is None:
        _NC = _build_bass()
    return _NC


def _host_prep(x, y_index, z, W1, b1, W2, b2):
    # flipped-kernel, tap-major/cout-minor per-branch stacks
    import ml_dtypes
    w1t = np.ascontiguousarray(
        W1[:, :, :, ::-1, ::-1].transpose(0, 1, 3, 4, 2)
    ).reshape(NB * C, 9, CSM).astype(ml_dtypes.bfloat16)
    w2t = np.ascontiguousarray(
        W2[:, :, :, ::-1, ::-1].transpose(0, 1, 3, 4, 2)
    ).reshape(NB * CSM, 9, C).astype(ml_dtypes.bfloat16)
    w1ga = np.zeros((NB * C, 9, 2 * CSM), dtype=ml_dtypes.bfloat16)
    w1gb = np.zeros_like(w1ga)
    w1ga[:, :, :CSM] = w1t
    w1gb[:, :, CSM:] = w1t
    w1ga, w1gb = w1ga.reshape(NB * C, -1), w1gb.reshape(NB * C, -1)
    # conv2 stacked tables: [dx-block 0..2] = dy rows; [dx-block 3..5] = dy2
    # (read only from the upper partition half). L tables feed rows 0:64
    # (dy=0 taps), U tables rows 64:128 (dy=1 taps + dy=2 taps).
    w2gl = np.zeros((2, NB * CSM, 6, 2 * C), dtype=ml_dtypes.bfloat16)
    w2gu = np.zeros((2, NB * CSM, 6, 2 * C), dtype=ml_dtypes.bfloat16)
    for s in range(2):
        cs = slice(s * C, (s + 1) * C)
        for dx in range(3):
            w2gl[s][:, dx, cs] = w2t[:, dx, :]
            w2gu[s][:, dx, cs] = w2t[:, 3 + dx, :]
            w2gu[s][:, 3 + dx, cs] = w2t[:, 6 + dx, :]
    w2gla, w2glb = w2gl[0].reshape(NB * CSM, -1), w2gl[1].reshape(NB * CSM, -1)
    w2gua, w2gub = w2gu[0].reshape(NB * CSM, -1), w2gu[1].reshape(NB * CSM, -1)
    b1g = b1.reshape(NB * CSM, 1).astype(np.float32)
    b2g = b2.reshape(NB * C, 1).astype(np.float32)

    in_maps = []
    for c in range(M):
        sl = slice(c * BS, (c + 1) * BS)
        in_maps.append(dict(
            xs=np.ascontiguousarray(x[sl]).reshape(BS * C, HW * HW).astype(np.float32),
            y32=y_index[sl].reshape(BS, 1).astype(np.int32),
            zs=np.ascontiguousarray(z[sl]).reshape(BS * C, 1).astype(np.float32),
            w1ga=w1ga, w1gb=w1gb, w2gla=w2gla, w2glb=w2glb,
            w2gua=w2gua, w2gub=w2gub, b1g=b1g, b2g=b2g,
        ))
    return in_maps


def kernel(x, y_index, y_hard, z, W1, b1, W2, b2, _trace=False):
    x = np.asarray(x, dtype=np.float32)
    z = np.asarray(z, dtype=np.float32)
    y_index = np.asarray(y_index)
    W1 = np.asarray(W1, dtype=np.float32)
    b1 = np.asarray(b1, dtype=np.float32)
    W2 = np.asarray(W2, dtype=np.float32)
    b2 = np.asarray(b2, dtype=np.float32)

    nc = _get_nc()
    in_maps = _host_prep(x, y_index, z, W1, b1, W2, b2)
    res = run_bass_kernel_spmd(nc, in_maps, core_ids=list(range(M)), trace=_trace)
    out = np.concatenate(
        [r["out"].reshape(BS, C, HW, HW) for r in res.results], axis=0
    )
    if _trace:
        kernel._last_results = res
    return out



# revision 2
# speedup vs baseline: 1.0458x; 1.0458x over previous
"""Trainium2 Bass kernel for nn_DecSwitchedDeconv — PE-array-tiled per-sample convs.

Strategy (data-parallel, 32 samples/core, groups of 8):
  - conv1 runs as 8 concurrent (64x32) PE tiles (2 row-groups x 4 col-groups):
    tile k = sample k of the group, 9 taps x 3 chunks of N=362 accumulate into
    2 PSUM banks ([128,362] = 4 samples' 32-cout slices each).
  - conv2 runs as 8 concurrent (32x64) PE tiles: bank r holds samples (r, r+4)
    as 64-cout halves.
  - All routing/gather work happens on host: per-sample weight stacks are
    gathered by y_index, kernels flipped/transposed, z folded into W2 and
    b2*z folded into the residual input xq = x + b2*z. On-chip epilogue is a
    single tensor_tensor add (psum + xq). relu(x) = relu(xq - b2z) via
    tensor_scalar on gpsimd; conv1 bias+relu via scalar ACT from PSUM.
  - I/O in bf16 (x-residual and output upcast on host), span layout (34-wide
    padded rows) so every op is a contiguous [128, N] slab.
"""

import numpy as np

import concourse.bacc as bacc
import concourse.bass as bass
import concourse.mybir as mybir
import concourse.tile as tile
from concourse.bass_utils import run_bass_kernel_spmd

B, C, CSM, NB, HW = 256, 64, 32, 8, 32
M = 8                    # cores
BS = B // M              # 32 samples/core
NG = BS // 8             # 4 groups of 8 samples
WP = HW + 2              # 34
L = WP * WP              # 1156
NVAL = (HW - 1) * WP + HW  # 1086 span covering all valid outputs
BASE = WP + 1            # 35
NCH = 3
CH = NVAL // NCH         # 362
NBUF = 2

f32 = mybir.dt.float32
bf16 = mybir.dt.bfloat16


def _build_bass():
    nc = bacc.Bacc(target_bir_lowering=False, debug=False)
    # per-core inputs (host pre-gathered/packed, all static)
    xqg = nc.dram_tensor("xqg", [NG * 128, 4 * NVAL], bf16, kind="ExternalInput")
    wg = nc.dram_tensor("wg", [NG * 128, 2304], bf16, kind="ExternalInput")
    b1g = nc.dram_tensor("b1g", [128, 2 * NG], f32, kind="ExternalInput")
    b2zg = nc.dram_tensor("b2zg", [128, 4 * NG], f32, kind="ExternalInput")
    outd = nc.dram_tensor("out", [NG * 4 * 128, HW * WP], bf16, kind="ExternalOutput")

    add = mybir.AluOpType.add
    Relu = mybir.ActivationFunctionType.Relu

    with tile.TileContext(nc) as tc:
        # persistent padded tensors; borders zeroed once and never rewritten
        xpads, hpas, hpbs = [], [], []
        for i in range(NBUF):
            xp = nc.alloc_sbuf_tensor(f"xpad{i}", [128, 4 * L], bf16).ap()
            ha = nc.alloc_sbuf_tensor(f"hpa{i}", [128, L], bf16).ap()
            hb = nc.alloc_sbuf_tensor(f"hpb{i}", [128, L], bf16).ap()
            nc.vector.memset(xp, 0.0)
            nc.gpsimd.memset(ha, 0.0)
            nc.gpsimd.memset(hb, 0.0)
            xpads.append(xp)
            hpas.append(ha)
            hpbs.append(hb)
        b1t = nc.alloc_sbuf_tensor("b1t", [128, 2 * NG], f32).ap()
        b2zt = nc.alloc_sbuf_tensor("b2zt", [128, 4 * NG], f32).ap()
        nc.sync.dma_start(b1t, b1g.ap())
        nc.sync.dma_start(b2zt, b2zg.ap())

        with (
            tc.tile_pool(name="io", bufs=3) as iop,
            tc.tile_pool(name="wp", bufs=3) as wpp,
            tc.tile_pool(name="osp", bufs=3) as ospp,
            tc.tile_pool(name="ps", bufs=8, space="PSUM") as psp,
        ):
            for g in range(NG):
                xpad = xpads[g % NBUF]
                ha, hb = hpas[g % NBUF], hpbs[g % NBUF]

                # ---- loads ----
                xqall = iop.tile([128, 4 * NVAL], bf16, tag="xq")
                nc.sync.dma_start(xqall[:, 0:2 * NVAL],
                                  xqg.ap()[g * 128:(g + 1) * 128, 0:2 * NVAL])
                nc.sync.dma_start(xqall[:, 2 * NVAL:4 * NVAL],
                                  xqg.ap()[g * 128:(g + 1) * 128, 2 * NVAL:4 * NVAL])
                xqs = [xqall[:, k * NVAL:(k + 1) * NVAL] for k in range(4)]
                wsb = wpp.tile([128, 2304], bf16, tag="w")
                nc.sync.dma_start(wsb[:, :], wg.ap()[g * 128:(g + 1) * 128, :])
                w1sb = wsb[:, 0:1152]
                w2sb = wsb[:, 1152:2304]

                # ---- relu(x) into padded blocks + junk-col rezero ----
                xp3 = xpad.rearrange("p (b h w) -> p b h w", b=4, w=WP)
                for k in range(4):
                    if k < 2:
                        nc.scalar.activation(
                            xpad[:, k * L + BASE:k * L + BASE + NVAL], xqs[k][:, :],
                            Relu, bias=b2zt[:, 4 * g + k:4 * g + k + 1])
                    else:
                        nc.vector.tensor_scalar(
                            xpad[:, k * L + BASE:k * L + BASE + NVAL], xqs[k][:, :],
                            b2zt[:, 4 * g + k:4 * g + k + 1], 0.0,
                            op0=add, op1=mybir.AluOpType.max)
                    nc.gpsimd.memset(xp3[:, k, 1:HW + 1, 0:WP:WP - 1], 0.0)

                # ---- conv1: 8 tiles of (64,32), 3 chunks x 9 taps ----
                for c in range(NCH):
                    psa = psp.tile([128, CH], f32, name=f"ps1a_{g}_{c}", tag="ps")
                    psb = psp.tile([128, CH], f32, name=f"ps1b_{g}_{c}", tag="ps")
                    for t in range(9):
                        dy, dx = divmod(t, 3)
                        off = dy * WP + dx + c * CH
                        for k in range(8):
                            row = 64 * (k // 4)
                            col = 32 * (k % 4)
                            ps = psa if k < 4 else psb
                            nc.tensor.matmul(
                                ps[col:col + 32, :],
                                lhsT=w1sb[row:row + 64,
                                          (k % 4) * 288 + t * 32:(k % 4) * 288 + (t + 1) * 32],
                                rhs=xpad[row:row + 64,
                                         (k % 4) * L + off:(k % 4) * L + off + CH],
                                start=(t == 0), stop=(t == 8),
                                tile_position=(row, col),
                            )
                    nc.scalar.activation(
                        ha[:, BASE + c * CH:BASE + (c + 1) * CH], psa[:, :],
                        Relu, bias=b1t[:, 2 * g:2 * g + 1])
                    nc.scalar.activation(
                        hb[:, BASE + c * CH:BASE + (c + 1) * CH], psb[:, :],
                        Relu, bias=b1t[:, 2 * g + 1:2 * g + 2])
                # junk-col rezero on h
                ha3 = ha.rearrange("p (h w) -> p h w", w=WP)
                hb3 = hb.rearrange("p (h w) -> p h w", w=WP)
                nc.gpsimd.memset(ha3[:, 1:HW + 1, 0:WP:WP - 1], 0.0)
                nc.gpsimd.memset(hb3[:, 1:HW + 1, 0:WP:WP - 1], 0.0)

                # ---- conv2: 8 tiles of (32,64) + fused residual epilogue ----
                outs = []
                for r in range(4):
                    osp = ospp.tile([128, 32 * WP], bf16, tag=f"os{r}")
                    outs.append(osp)
                for c in range(NCH):
                    pss = [psp.tile([128, CH], f32, name=f"ps2_{g}_{c}_{r}", tag="ps")
                           for r in range(4)]
                    for t in range(9):
                        dy, dx = divmod(t, 3)
                        off = dy * WP + dx + c * CH
                        for k in range(8):
                            r, h = divmod(k, 2)
                            src = ha if h == 0 else hb
                            nc.tensor.matmul(
                                pss[r][64 * h:64 * h + 64, :],
                                lhsT=w2sb[32 * r:32 * r + 32,
                                          h * 576 + t * 64:h * 576 + (t + 1) * 64],
                                rhs=src[32 * r:32 * r + 32, off:off + CH],
                                start=(t == 0), stop=(t == 8),
                                tile_position=(32 * r, 64 * h),
                            )
                    for r in range(4):
                        nc.vector.tensor_tensor(
                            outs[r][:, c * CH:(c + 1) * CH], pss[r][:, :],
                            xqs[r][:, c * CH:(c + 1) * CH], op=add)

                # ---- store (full 1088-col span rows; junk stripped on host) ----
                for r in range(4):
                    nc.gpsimd.dma_start(
                        outd.ap()[(g * 4 + r) * 128:(g * 4 + r + 1) * 128, :],
                        outs[r][:, :])

    nc.compile()
    return nc


_NC = None


def _get_nc():
    global _NC
    if _NC is None:
        _NC = _build_bass()
    return _NC


def _host_prep(x, y_index, z, W1, b1, W2, b2):
    import ml_dtypes
    idx = np.asarray(y_index).reshape(B).astype(np.int64)
    # flipped-kernel stacks: w1t [NB, C, 9, CSM], w2t [NB, CSM, 9, C]
    w1t = np.ascontiguousarray(
        W1[:, :, :, ::-1, ::-1].transpose(0, 1, 3, 4, 2)).reshape(NB, C, 9, CSM)
    w2t = np.ascontiguousarray(
        W2[:, :, :, ::-1, ::-1].transpose(0, 1, 3, 4, 2)).reshape(NB, CSM, 9, C)
    w1s = w1t[idx]                                   # [B, 64, 9, 32] f32
    w2s = w2t[idx] * z[:, None, None, :]             # [B, 32, 9, 64] f32
    b2z = b2[idx] * z                                # [B, 64]
    b1s = b1[idx]                                    # [B, 32]

    # xq = x + b2z, span layout [B, 64, 1086] with zeros at junk cols
    xq = x + b2z[:, :, None, None]
    xqp = np.zeros((B, C, WP, WP), np.float32)
    xqp[:, :, 1:HW + 1, 1:HW + 1] = xq
    xq_span = xqp.reshape(B, C, L)[:, :, BASE:BASE + NVAL]
    xq_span = xq_span.astype(ml_dtypes.bfloat16)

    w1sb = w1s.astype(ml_dtypes.bfloat16)
    w2sb = w2s.astype(ml_dtypes.bfloat16)

    in_maps = []
    for cr in range(M):
        s0 = cr * BS
        # xqg rows: (g, k) pair tile = samples (s0+8g+k | s0+8g+4+k)
        xqg = np.empty((NG * 128, 4 * NVAL), ml_dtypes.bfloat16)
        wgh = np.zeros((NG * 128, 2304), ml_dtypes.bfloat16)
        b1h = np.zeros((128, 2 * NG), np.float32)
        b2zh = np.zeros((128, 4 * NG), np.float32)
        for g in range(NG):
            for k in range(4):
                sa, sb = s0 + 8 * g + k, s0 + 8 * g + 4 + k
                r0 = g * 128
                xqg[r0:r0 + 64, k * NVAL:(k + 1) * NVAL] = xq_span[sa]
                xqg[r0 + 64:r0 + 128, k * NVAL:(k + 1) * NVAL] = xq_span[sb]
                b2zh[0:64, 4 * g + k] = -b2z[sa]
                b2zh[64:128, 4 * g + k] = b2z[sb]
                # conv1 weights: tile k (cols k*288) top=sa, tile 8+k bottom=sb
                wgh[g * 128:g * 128 + 64,
                    k * 288:(k + 1) * 288] = w1s[sa].reshape(64, 288)
                wgh[g * 128 + 64:(g + 1) * 128,
                    k * 288:(k + 1) * 288] = w1s[sb].reshape(64, 288)
                # conv1 bias: bank a (cols 2g) = samples sa at 32*k..; bank b = sb
                b1h[32 * k:32 * (k + 1), 2 * g] = b1s[sa]
                b1h[32 * k:32 * (k + 1), 2 * g + 1] = b1s[sb]
                # conv2 weights: tile (32k, 64h): h=0 -> sa, h=1 -> sb
                wgh[g * 128 + 32 * k:g * 128 + 32 * (k + 1),
                    1152:1728] = w2sb[sa].reshape(32, 576)
                wgh[g * 128 + 32 * k:g * 128 + 32 * (k + 1),
                    1728:2304] = w2sb[sb].reshape(32, 576)
        in_maps.append(dict(xqg=xqg, wg=wgh, b1g=b1h, b2zg=b2zh))
    return in_maps


def kernel(x, y_index, y_hard, z, W1, b1, W2, b2, _trace=False):
    x = np.asarray(x, dtype=np.float32)
    z = np.asarray(z, dtype=np.float32)
    W1 = np.asarray(W1, dtype=np.float32)
    b1 = np.asarray(b1, dtype=np.float32)
    W2 = np.asarray(W2, dtype=np.float32)
    b2 = np.asarray(b2, dtype=np.float32)

    nc = _get_nc()
    in_maps = _host_prep(x, y_index, z, W1, b1, W2, b2)
    res = run_bass_kernel_spmd(nc, in_maps, core_ids=list(range(M)), trace=_trace)
    out = np.empty((B, C, HW, HW), np.float32)
    for cr in range(M):
        o = np.asarray(res.results[cr]["out"], dtype=np.float32)
        o = o.reshape(NG, 4, 2, C, HW, WP)[..., 0:HW]  # strip junk cols
        for g in range(NG):
            for k in range(4):
                out[cr * BS + 8 * g + k] = o[g, k, 0]
                out[cr * BS + 8 * g + 4 + k] = o[g, k, 1]
    if _trace:
        kernel._last_results = res
    return out


# revision 3
# speedup vs baseline: 1.0470x; 1.0012x over previous
"""Trainium2 Bass kernel for nn_DecSwitchedDeconv — PE-array-tiled per-sample convs.

Strategy (data-parallel, 32 samples/core, groups of 8):
  - conv1 runs as 8 concurrent (64x32) PE tiles (2 row-groups x 4 col-groups):
    tile k = sample k of the group, 9 taps x 3 chunks of N=362 accumulate into
    2 PSUM banks ([128,362] = 4 samples' 32-cout slices each).
  - conv2 runs as 8 concurrent (32x64) PE tiles: bank r holds samples (r, r+4)
    as 64-cout halves.
  - All routing/gather work happens on host: per-sample weight stacks are
    gathered by y_index, kernels flipped/transposed, z folded into W2 and
    b2*z folded into the residual input xq = x + b2*z. On-chip epilogue is a
    single tensor_tensor add (psum + xq). relu(x) = relu(xq - b2z) via
    tensor_scalar on gpsimd; conv1 bias+relu via scalar ACT from PSUM.
  - I/O in bf16 (x-residual and output upcast on host), span layout (34-wide
    padded rows) so every op is a contiguous [128, N] slab.
"""

import numpy as np

import concourse.bacc as bacc
import concourse.bass as bass
import concourse.mybir as mybir
import concourse.tile as tile
from concourse.bass_utils import run_bass_kernel_spmd

B, C, CSM, NB, HW = 256, 64, 32, 8, 32
M = 8                    # cores
BS = B // M              # 32 samples/core
NG = BS // 8             # 4 groups of 8 samples
WP = HW + 2              # 34
L = WP * WP              # 1156
NVAL = (HW - 1) * WP + HW  # 1086 span covering all valid outputs
BASE = WP + 1            # 35
NCH = 3
CH = NVAL // NCH         # 362
NBUF = 2

f32 = mybir.dt.float32
bf16 = mybir.dt.bfloat16


def _build_bass():
    nc = bacc.Bacc(target_bir_lowering=False, debug=False)
    # per-core inputs (host pre-gathered/packed, all static)
    xqg = nc.dram_tensor("xqg", [NG * 128, 4 * NVAL], bf16, kind="ExternalInput")
    wg = nc.dram_tensor("wg", [NG * 128, 2304], bf16, kind="ExternalInput")
    b1g = nc.dram_tensor("b1g", [128, 2 * NG], f32, kind="ExternalInput")
    b2zg = nc.dram_tensor("b2zg", [128, 4 * NG], f32, kind="ExternalInput")
    outd = nc.dram_tensor("out", [NG * 4 * 128, HW * WP], bf16, kind="ExternalOutput")

    add = mybir.AluOpType.add
    Relu = mybir.ActivationFunctionType.Relu

    with tile.TileContext(nc) as tc:
        # persistent padded tensors; borders zeroed once and never rewritten
        xpads, hpas, hpbs = [], [], []
        for i in range(NBUF):
            xp = nc.alloc_sbuf_tensor(f"xpad{i}", [128, 4 * L], bf16).ap()
            ha = nc.alloc_sbuf_tensor(f"hpa{i}", [128, L], bf16).ap()
            hb = nc.alloc_sbuf_tensor(f"hpb{i}", [128, L], bf16).ap()
            for k in range(4):
                nc.vector.memset(xp[:, k * L:k * L + BASE], 0.0)
                nc.vector.memset(xp[:, k * L + BASE + NVAL:(k + 1) * L], 0.0)
            nc.gpsimd.memset(ha[:, 0:BASE], 0.0)
            nc.gpsimd.memset(ha[:, BASE + NVAL:L], 0.0)
            nc.gpsimd.memset(hb[:, 0:BASE], 0.0)
            nc.gpsimd.memset(hb[:, BASE + NVAL:L], 0.0)
            xpads.append(xp)
            hpas.append(ha)
            hpbs.append(hb)
        warm_sb = nc.alloc_sbuf_tensor("warm_sb", [128, 640], bf16).ap()
        b1t = nc.alloc_sbuf_tensor("b1t", [128, 2 * NG], f32).ap()
        b2zt = nc.alloc_sbuf_tensor("b2zt", [128, 4 * NG], f32).ap()
        nc.sync.dma_start(b2zt, b2zg.ap())

        with (
            tc.tile_pool(name="io", bufs=3) as iop,
            tc.tile_pool(name="wp", bufs=3) as wpp,
            tc.tile_pool(name="osp", bufs=3) as ospp,
            tc.tile_pool(name="ps", bufs=8, space="PSUM") as psp,
        ):
            # HAM pre-warm: dummy full-array matmuls while first loads land
            wps = psp.tile([128, CH], f32, name="warm_ps", tag="ps")
            for i in range(28):
                nc.tensor.matmul(
                    wps[:, 0:256], lhsT=warm_sb[:, 512:640],
                    rhs=warm_sb[:, 0:256], start=True, stop=True)

            def emit_loads(g):
                xqall = iop.tile([128, 4 * NVAL], bf16, tag="xq", name=f"xq_{g}")
                wsb = wpp.tile([128, 2304], bf16, tag="w", name=f"w_{g}")
                nc.gpsimd.dma_start(wsb[:, :], wg.ap()[g * 128:(g + 1) * 128, :])
                if g == 0:
                    for k in range(4):
                        nc.sync.dma_start(
                            xqall[:, k * NVAL:(k + 1) * NVAL],
                            xqg.ap()[g * 128:(g + 1) * 128, k * NVAL:(k + 1) * NVAL])
                    nc.sync.dma_start(b1t, b1g.ap())
                else:
                    nc.sync.dma_start(xqall[:, 0:2 * NVAL],
                                      xqg.ap()[g * 128:(g + 1) * 128, 0:2 * NVAL])
                    nc.sync.dma_start(xqall[:, 2 * NVAL:4 * NVAL],
                                      xqg.ap()[g * 128:(g + 1) * 128, 2 * NVAL:4 * NVAL])
                return xqall, wsb

            pending = emit_loads(0)
            for g in range(NG):
                xpad = xpads[g % NBUF]
                ha, hb = hpas[g % NBUF], hpbs[g % NBUF]
                xqall, wsb = pending
                xqs = [xqall[:, k * NVAL:(k + 1) * NVAL] for k in range(4)]
                w1sb = wsb[:, 0:1152]
                w2sb = wsb[:, 1152:2304]

                # ---- relu(x) into padded blocks + junk-col rezero ----
                xp3 = xpad.rearrange("p (b h w) -> p b h w", b=4, w=WP)
                for k in range(4):
                    if g == 0:
                        nc.vector.tensor_scalar(
                            xpad[:, k * L + BASE:k * L + BASE + NVAL], xqs[k][:, :],
                            b2zt[:, 4 * g + k:4 * g + k + 1], 0.0,
                            op0=add, op1=mybir.AluOpType.max)
                    else:
                        nc.scalar.activation(
                            xpad[:, k * L + BASE:k * L + BASE + NVAL], xqs[k][:, :],
                            Relu, bias=b2zt[:, 4 * g + k:4 * g + k + 1])
                    nc.gpsimd.memset(xp3[:, k, 1:HW + 1, 0:WP:WP - 1], 0.0)

                # ---- conv1: 8 tiles of (64,32), 3 chunks x 9 taps ----
                for c in range(NCH):
                    psa = psp.tile([128, CH], f32, name=f"ps1a_{g}_{c}", tag="ps")
                    psb = psp.tile([128, CH], f32, name=f"ps1b_{g}_{c}", tag="ps")
                    for t in range(9):
                        dy, dx = divmod(t, 3)
                        off = dy * WP + dx + c * CH
                        for k in range(8):
                            row = 64 * (k // 4)
                            col = 32 * (k % 4)
                            ps = psa if k < 4 else psb
                            nc.tensor.matmul(
                                ps[col:col + 32, :],
                                lhsT=w1sb[row:row + 64,
                                          (k % 4) * 288 + t * 32:(k % 4) * 288 + (t + 1) * 32],
                                rhs=xpad[row:row + 64,
                                         (k % 4) * L + off:(k % 4) * L + off + CH],
                                start=(t == 0), stop=(t == 8),
                                tile_position=(row, col),
                            )
                    nc.vector.tensor_scalar(
                        ha[:, BASE + c * CH:BASE + (c + 1) * CH], psa[:, :],
                        b1t[:, 2 * g:2 * g + 1], 0.0,
                        op0=add, op1=mybir.AluOpType.max)
                    nc.vector.tensor_scalar(
                        hb[:, BASE + c * CH:BASE + (c + 1) * CH], psb[:, :],
                        b1t[:, 2 * g + 1:2 * g + 2], 0.0,
                        op0=add, op1=mybir.AluOpType.max)
                # junk-col rezero on h
                ha3 = ha.rearrange("p (h w) -> p h w", w=WP)
                hb3 = hb.rearrange("p (h w) -> p h w", w=WP)
                nc.gpsimd.memset(ha3[:, 1:HW + 1, 0:WP:WP - 1], 0.0)
                nc.gpsimd.memset(hb3[:, 1:HW + 1, 0:WP:WP - 1], 0.0)

                if g + 1 < NG:
                    pending = emit_loads(g + 1)

                # ---- conv2: 8 tiles of (32,64) + fused residual epilogue ----
                outs = []
                for r in range(4):
                    osp = ospp.tile([128, 32 * WP], bf16, tag=f"os{r}")
                    outs.append(osp)
                for c in range(NCH):
                    pss = [psp.tile([128, CH], f32, name=f"ps2_{g}_{c}_{r}", tag="ps")
                           for r in range(4)]
                    for t in range(9):
                        dy, dx = divmod(t, 3)
                        off = dy * WP + dx + c * CH
                        for k in range(8):
                            r, h = divmod(k, 2)
                            src = ha if h == 0 else hb
                            nc.tensor.matmul(
                                pss[r][64 * h:64 * h + 64, :],
                                lhsT=w2sb[32 * r:32 * r + 32,
                                          h * 576 + t * 64:h * 576 + (t + 1) * 64],
                                rhs=src[32 * r:32 * r + 32, off:off + CH],
                                start=(t == 0), stop=(t == 8),
                                tile_position=(32 * r, 64 * h),
                            )
                    for r in range(4):
                        nc.vector.tensor_tensor(
                            outs[r][:, c * CH:(c + 1) * CH], pss[r][:, :],
                            xqs[r][:, c * CH:(c + 1) * CH], op=add)
                        eng = nc.sync if r % 2 == 0 else nc.gpsimd
                        eng.dma_start(
                            outd.ap()[(g * 4 + r) * 128:(g * 4 + r + 1) * 128,
                                      c * CH:(c + 1) * CH],
                            outs[r][:, c * CH:(c + 1) * CH])


    nc.compile()
    return nc


import os as _os
if _os.environ.get("LDWOPT", "0") == "1":
    import concourse.bass_utils as _bu
    if not getattr(_bu, "_ldw_patched", False):
        _orig = _bu.run_command
        def _rc(argv, **kw):
            argv = ["--enable-ldw-opt=true" if a == "--enable-ldw-opt=false" else a
                    for a in argv]
            return _orig(argv, **kw)
        _bu.run_command = _rc
        _bu._ldw_patched = True

_NC = None


def _get_nc():
    global _NC
    if _NC is None:
        _NC = _build_bass()
    return _NC


def _host_prep(x, y_index, z, W1, b1, W2, b2):
    import ml_dtypes
    idx = np.asarray(y_index).reshape(B).astype(np.int64)
    # flipped-kernel stacks: w1t [NB, C, 9, CSM], w2t [NB, CSM, 9, C]
    w1t = np.ascontiguousarray(
        W1[:, :, :, ::-1, ::-1].transpose(0, 1, 3, 4, 2)).reshape(NB, C, 9, CSM)
    w2t = np.ascontiguousarray(
        W2[:, :, :, ::-1, ::-1].transpose(0, 1, 3, 4, 2)).reshape(NB, CSM, 9, C)
    w1s = w1t[idx]                                   # [B, 64, 9, 32] f32
    w2s = w2t[idx] * z[:, None, None, :]             # [B, 32, 9, 64] f32
    b2z = b2[idx] * z                                # [B, 64]
    b1s = b1[idx]                                    # [B, 32]

    # xq = x + b2z, span layout [B, 64, 1086] with zeros at junk cols
    xq = x + b2z[:, :, None, None]
    xqp = np.zeros((B, C, WP, WP), np.float32)
    xqp[:, :, 1:HW + 1, 1:HW + 1] = xq
    xq_span = xqp.reshape(B, C, L)[:, :, BASE:BASE + NVAL].copy()
    ji = np.array([i for i in range(NVAL) if (BASE + i) % WP in (0, WP - 1)])
    xq_span[:, :, ji] = -1e30
    xq_span = xq_span.astype(ml_dtypes.bfloat16)

    w1sb = w1s.astype(ml_dtypes.bfloat16)
    w2sb = w2s.astype(ml_dtypes.bfloat16)

    in_maps = []
    for cr in range(M):
        s0 = cr * BS
        # xqg rows: (g, k) pair tile = samples (s0+8g+k | s0+8g+4+k)
        xqg = np.empty((NG * 128, 4 * NVAL), ml_dtypes.bfloat16)
        wgh = np.zeros((NG * 128, 2304), ml_dtypes.bfloat16)
        b1h = np.zeros((128, 2 * NG), np.float32)
        b2zh = np.zeros((128, 4 * NG), np.float32)
        for g in range(NG):
            for k in range(4):
                sa, sb = s0 + 8 * g + k, s0 + 8 * g + 4 + k
                r0 = g * 128
                xqg[r0:r0 + 64, k * NVAL:(k + 1) * NVAL] = xq_span[sa]
                xqg[r0 + 64:r0 + 128, k * NVAL:(k + 1) * NVAL] = xq_span[sb]
                b2zh[0:64, 4 * g + k] = -b2z[sa]
                b2zh[64:128, 4 * g + k] = b2z[sb]
                # conv1 weights: tile k (cols k*288) top=sa, tile 8+k bottom=sb
                wgh[g * 128:g * 128 + 64,
                    k * 288:(k + 1) * 288] = w1s[sa].reshape(64, 288)
                wgh[g * 128 + 64:(g + 1) * 128,
                    k * 288:(k + 1) * 288] = w1s[sb].reshape(64, 288)
                # conv1 bias: bank a (cols 2g) = samples sa at 32*k..; bank b = sb
                b1h[32 * k:32 * (k + 1), 2 * g] = b1s[sa]
                b1h[32 * k:32 * (k + 1), 2 * g + 1] = b1s[sb]
                # conv2 weights: tile (32k, 64h): h=0 -> sa, h=1 -> sb
                wgh[g * 128 + 32 * k:g * 128 + 32 * (k + 1),
                    1152:1728] = w2sb[sa].reshape(32, 576)
                wgh[g * 128 + 32 * k:g * 128 + 32 * (k + 1),
                    1728:2304] = w2sb[sb].reshape(32, 576)
        in_maps.append(dict(xqg=xqg, wg=wgh, b1g=b1h, b2zg=b2zh))
    return in_maps


def kernel(x, y_index, y_hard, z, W1, b1, W2, b2, _trace=False):
    x = np.asarray(x, dtype=np.float32)
    z = np.asarray(z, dtype=np.float32)
    W1 = np.asarray(W1, dtype=np.float32)
    b1 = np.asarray(b1, dtype=np.float32)
    W2 = np.asarray(W2, dtype=np.float32)
    b2 = np.asarray(b2, dtype=np.float32)

    nc = _get_nc()
    in_maps = _host_prep(x, y_index, z, W1, b1, W2, b2)
    res = run_bass_kernel_spmd(nc, in_maps, core_ids=list(range(M)), trace=_trace)
    out = np.empty((B, C, HW, HW), np.float32)
    for cr in range(M):
        o = np.asarray(res.results[cr]["out"], dtype=np.float32)
        o = o.reshape(NG, 4, 2, C, HW, WP)[..., 0:HW]  # strip junk cols
        for g in range(NG):
            for k in range(4):
                out[cr * BS + 8 * g + k] = o[g, k, 0]
                out[cr * BS + 8 * g + 4 + k] = o[g, k, 1]
    if _trace:
        kernel._last_results = res
    return out


# revision 4
# speedup vs baseline: 1.0815x; 1.0330x over previous
"""Trainium2 Bass kernel for nn_DecSwitchedDeconv — PE-array-tiled per-sample convs.

Strategy (data-parallel, 32 samples/core, groups of 8):
  - conv1 runs as 8 concurrent (64x32) PE tiles (2 row-groups x 4 col-groups):
    tile k = sample k of the group, 9 taps x 3 chunks of N=362 accumulate into
    2 PSUM banks ([128,362] = 4 samples' 32-cout slices each).
  - conv2 runs as 8 concurrent (32x64) PE tiles: bank r holds samples (r, r+4)
    as 64-cout halves.
  - All routing/gather work happens on host: per-sample weight stacks are
    gathered by y_index, kernels flipped/transposed, z folded into W2 and
    b2*z folded into the residual input xq = x + b2*z. On-chip epilogue is a
    single vector tensor_tensor add (psum + xq). relu(x) = relu(xq - b2z) via
    scalar ACT bias (vector tensor_scalar for group 0, dodging the ACT-table
    load latency); conv1 bias+relu evac via vector tensor_scalar so PSUM
    release order on the vector queue matches the scheduler's ready order.
  - One shared PSUM pool (single tag, bufs=8) rotates all 8 banks through
    conv1 (2 tiles/chunk) and conv2 (4 tiles/chunk) for free double-buffering.
  - Junk columns between padded rows are painted -1e30 in xq on host so the
    relu writes zeros there (no on-chip fixup memsets for xpad); h junk cols
    are re-zeroed with tiny strided memsets.
  - 28 dummy full-array matmuls on scratch SBUF pre-warm the PE clock (HAM)
    while the first loads land; loads prefetch one group ahead on sync/gpsimd
    queues, stores split across both.
  - I/O in bf16 (x-residual and output upcast on host), span layout (34-wide
    padded rows) so every op is a contiguous [128, N] slab.
"""

import numpy as np

import concourse.bacc as bacc
import concourse.bass as bass
import concourse.mybir as mybir
import concourse.tile as tile
from concourse.bass_utils import run_bass_kernel_spmd

B, C, CSM, NB, HW = 256, 64, 32, 8, 32
M = 8                    # cores
BS = B // M              # 32 samples/core
NG = BS // 8             # 4 groups of 8 samples
WP = HW + 2              # 34
L = WP * WP              # 1156
NVAL = (HW - 1) * WP + HW  # 1086 span covering all valid outputs
BASE = WP + 1            # 35
NCH = 3
CH = NVAL // NCH         # 362
NBUF = 2

f32 = mybir.dt.float32
bf16 = mybir.dt.bfloat16


def _build_bass():
    nc = bacc.Bacc(target_bir_lowering=False, debug=False)
    # per-core inputs (host pre-gathered/packed, all static)
    xqg = nc.dram_tensor("xqg", [NG * 128, 4 * NVAL], bf16, kind="ExternalInput")
    wg = nc.dram_tensor("wg", [NG * 128, 2304], bf16, kind="ExternalInput")
    b1g = nc.dram_tensor("b1g", [128, 2 * NG], f32, kind="ExternalInput")
    b2zg = nc.dram_tensor("b2zg", [128, 4 * NG], f32, kind="ExternalInput")
    outd = nc.dram_tensor("out", [NG * 4 * 128, HW * WP], bf16, kind="ExternalOutput")

    add = mybir.AluOpType.add
    Relu = mybir.ActivationFunctionType.Relu

    with tile.TileContext(nc) as tc:
        # persistent padded tensors; borders zeroed once and never rewritten
        xpads, hpas, hpbs = [], [], []
        for i in range(NBUF):
            xp = nc.alloc_sbuf_tensor(f"xpad{i}", [128, 4 * L], bf16).ap()
            ha = nc.alloc_sbuf_tensor(f"hpa{i}", [128, L], bf16).ap()
            hb = nc.alloc_sbuf_tensor(f"hpb{i}", [128, L], bf16).ap()
            for k in range(4):
                nc.vector.memset(xp[:, k * L:k * L + BASE], 0.0)
                nc.vector.memset(xp[:, k * L + BASE + NVAL:(k + 1) * L], 0.0)
            nc.gpsimd.memset(ha[:, 0:BASE], 0.0)
            nc.gpsimd.memset(ha[:, BASE + NVAL:L], 0.0)
            nc.gpsimd.memset(hb[:, 0:BASE], 0.0)
            nc.gpsimd.memset(hb[:, BASE + NVAL:L], 0.0)
            xpads.append(xp)
            hpas.append(ha)
            hpbs.append(hb)
        warm_sb = nc.alloc_sbuf_tensor("warm_sb", [128, 640], bf16).ap()
        b1t = nc.alloc_sbuf_tensor("b1t", [128, 2 * NG], f32).ap()
        b2zt = nc.alloc_sbuf_tensor("b2zt", [128, 4 * NG], f32).ap()
        nc.sync.dma_start(b2zt, b2zg.ap())

        with (
            tc.tile_pool(name="io", bufs=3) as iop,
            tc.tile_pool(name="wp", bufs=3) as wpp,
            tc.tile_pool(name="osp", bufs=3) as ospp,
            tc.tile_pool(name="ps", bufs=8, space="PSUM") as psp,
        ):
            # HAM pre-warm: dummy full-array matmuls while first loads land
            wps = psp.tile([128, CH], f32, name="warm_ps", tag="ps")
            for i in range(28):
                nc.tensor.matmul(
                    wps[:, 0:256], lhsT=warm_sb[:, 512:640],
                    rhs=warm_sb[:, 0:256], start=True, stop=True)

            def emit_loads(g):
                xqall = iop.tile([128, 4 * NVAL], bf16, tag="xq", name=f"xq_{g}")
                wsb = wpp.tile([128, 2304], bf16, tag="w", name=f"w_{g}")
                nc.gpsimd.dma_start(wsb[:, :], wg.ap()[g * 128:(g + 1) * 128, :])
                if g == 0:
                    for k in range(4):
                        nc.sync.dma_start(
                            xqall[:, k * NVAL:(k + 1) * NVAL],
                            xqg.ap()[g * 128:(g + 1) * 128, k * NVAL:(k + 1) * NVAL])
                    nc.sync.dma_start(b1t, b1g.ap())
                else:
                    nc.sync.dma_start(xqall[:, 0:2 * NVAL],
                                      xqg.ap()[g * 128:(g + 1) * 128, 0:2 * NVAL])
                    nc.sync.dma_start(xqall[:, 2 * NVAL:4 * NVAL],
                                      xqg.ap()[g * 128:(g + 1) * 128, 2 * NVAL:4 * NVAL])
                return xqall, wsb

            pending = emit_loads(0)
            for g in range(NG):
                xpad = xpads[g % NBUF]
                ha, hb = hpas[g % NBUF], hpbs[g % NBUF]
                xqall, wsb = pending
                xqs = [xqall[:, k * NVAL:(k + 1) * NVAL] for k in range(4)]
                w1sb = wsb[:, 0:1152]
                w2sb = wsb[:, 1152:2304]

                # ---- relu(x) into padded blocks + junk-col rezero ----
                xp3 = xpad.rearrange("p (b h w) -> p b h w", b=4, w=WP)
                for k in range(4):
                    if g == 0:
                        nc.vector.tensor_scalar(
                            xpad[:, k * L + BASE:k * L + BASE + NVAL], xqs[k][:, :],
                            b2zt[:, 4 * g + k:4 * g + k + 1], 0.0,
                            op0=add, op1=mybir.AluOpType.max)
                    else:
                        nc.scalar.activation(
                            xpad[:, k * L + BASE:k * L + BASE + NVAL], xqs[k][:, :],
                            Relu, bias=b2zt[:, 4 * g + k:4 * g + k + 1])
                    nc.gpsimd.memset(xp3[:, k, 1:HW + 1, 0:WP:WP - 1], 0.0)

                # ---- conv1: 8 tiles of (64,32), 3 chunks x 9 taps ----
                for c in range(NCH):
                    psa = psp.tile([128, CH], f32, name=f"ps1a_{g}_{c}", tag="ps")
                    psb = psp.tile([128, CH], f32, name=f"ps1b_{g}_{c}", tag="ps")
                    for t in range(9):
                        dy, dx = divmod(t, 3)
                        off = dy * WP + dx + c * CH
                        for k in range(8):
                            row = 64 * (k // 4)
                            col = 32 * (k % 4)
                            ps = psa if k < 4 else psb
                            nc.tensor.matmul(
                                ps[col:col + 32, :],
                                lhsT=w1sb[row:row + 64,
                                          (k % 4) * 288 + t * 32:(k % 4) * 288 + (t + 1) * 32],
                                rhs=xpad[row:row + 64,
                                         (k % 4) * L + off:(k % 4) * L + off + CH],
                                start=(t == 0), stop=(t == 8),
                                tile_position=(row, col),
                            )
                    nc.vector.tensor_scalar(
                        ha[:, BASE + c * CH:BASE + (c + 1) * CH], psa[:, :],
                        b1t[:, 2 * g:2 * g + 1], 0.0,
                        op0=add, op1=mybir.AluOpType.max)
                    nc.vector.tensor_scalar(
                        hb[:, BASE + c * CH:BASE + (c + 1) * CH], psb[:, :],
                        b1t[:, 2 * g + 1:2 * g + 2], 0.0,
                        op0=add, op1=mybir.AluOpType.max)
                # junk-col rezero on h
                ha3 = ha.rearrange("p (h w) -> p h w", w=WP)
                hb3 = hb.rearrange("p (h w) -> p h w", w=WP)
                nc.gpsimd.memset(ha3[:, 1:HW + 1, 0:WP:WP - 1], 0.0)
                nc.gpsimd.memset(hb3[:, 1:HW + 1, 0:WP:WP - 1], 0.0)

                if g + 1 < NG:
                    pending = emit_loads(g + 1)

                # ---- conv2: 8 tiles of (32,64) + fused residual epilogue ----
                outs = []
                for r in range(4):
                    osp = ospp.tile([128, 32 * WP], bf16, tag=f"os{r}")
                    outs.append(osp)
                for c in range(NCH):
                    pss = [psp.tile([128, CH], f32, name=f"ps2_{g}_{c}_{r}", tag="ps")
                           for r in range(4)]
                    for t in range(9):
                        dy, dx = divmod(t, 3)
                        off = dy * WP + dx + c * CH
                        for k in range(8):
                            r, h = divmod(k, 2)
                            src = ha if h == 0 else hb
                            nc.tensor.matmul(
                                pss[r][64 * h:64 * h + 64, :],
                                lhsT=w2sb[32 * r:32 * r + 32,
                                          h * 576 + t * 64:h * 576 + (t + 1) * 64],
                                rhs=src[32 * r:32 * r + 32, off:off + CH],
                                start=(t == 0), stop=(t == 8),
                                tile_position=(32 * r, 64 * h),
                            )
                    for r in range(4):
                        nc.vector.tensor_tensor(
                            outs[r][:, c * CH:(c + 1) * CH], pss[r][:, :],
                            xqs[r][:, c * CH:(c + 1) * CH], op=add)
                        eng = nc.sync if r % 2 == 0 else nc.gpsimd
                        eng.dma_start(
                            outd.ap()[(g * 4 + r) * 128:(g * 4 + r + 1) * 128,
                                      c * CH:(c + 1) * CH],
                            outs[r][:, c * CH:(c + 1) * CH])


    nc.compile()
    return nc


import os as _os
if _os.environ.get("LDWOPT", "0") == "1":
    import concourse.bass_utils as _bu
    if not getattr(_bu, "_ldw_patched", False):
        _orig = _bu.run_command
        def _rc(argv, **kw):
            argv = ["--enable-ldw-opt=true" if a == "--enable-ldw-opt=false" else a
                    for a in argv]
            return _orig(argv, **kw)
        _bu.run_command = _rc
        _bu._ldw_patched = True

_NC = None


def _get_nc():
    global _NC
    if _NC is None:
        _NC = _build_bass()
    return _NC


def _host_prep(x, y_index, z, W1, b1, W2, b2):
    import ml_dtypes
    idx = np.asarray(y_index).reshape(B).astype(np.int64)
    # flipped-kernel stacks: w1t [NB, C, 9, CSM], w2t [NB, CSM, 9, C]
    w1t = np.ascontiguousarray(
        W1[:, :, :, ::-1, ::-1].transpose(0, 1, 3, 4, 2)).reshape(NB, C, 9, CSM)
    w2t = np.ascontiguousarray(
        W2[:, :, :, ::-1, ::-1].transpose(0, 1, 3, 4, 2)).reshape(NB, CSM, 9, C)
    w1s = w1t[idx]                                   # [B, 64, 9, 32] f32
    w2s = w2t[idx] * z[:, None, None, :]             # [B, 32, 9, 64] f32
    b2z = b2[idx] * z                                # [B, 64]
    b1s = b1[idx]                                    # [B, 32]

    # xq = x + b2z, span layout [B, 64, 1086] with zeros at junk cols
    xq = x + b2z[:, :, None, None]
    xqp = np.zeros((B, C, WP, WP), np.float32)
    xqp[:, :, 1:HW + 1, 1:HW + 1] = xq
    xq_span = xqp.reshape(B, C, L)[:, :, BASE:BASE + NVAL].copy()
    ji = np.array([i for i in range(NVAL) if (BASE + i) % WP in (0, WP - 1)])
    xq_span[:, :, ji] = -1e30
    xq_span = xq_span.astype(ml_dtypes.bfloat16)

    w1sb = w1s.astype(ml_dtypes.bfloat16)
    w2sb = w2s.astype(ml_dtypes.bfloat16)

    in_maps = []
    for cr in range(M):
        s0 = cr * BS
        # xqg rows: (g, k) pair tile = samples (s0+8g+k | s0+8g+4+k)
        xqg = np.empty((NG * 128, 4 * NVAL), ml_dtypes.bfloat16)
        wgh = np.zeros((NG * 128, 2304), ml_dtypes.bfloat16)
        b1h = np.zeros((128, 2 * NG), np.float32)
        b2zh = np.zeros((128, 4 * NG), np.float32)
        for g in range(NG):
            for k in range(4):
                sa, sb = s0 + 8 * g + k, s0 + 8 * g + 4 + k
                r0 = g * 128
                xqg[r0:r0 + 64, k * NVAL:(k + 1) * NVAL] = xq_span[sa]
                xqg[r0 + 64:r0 + 128, k * NVAL:(k + 1) * NVAL] = xq_span[sb]
                b2zh[0:64, 4 * g + k] = -b2z[sa]
                b2zh[64:128, 4 * g + k] = b2z[sb]
                # conv1 weights: tile k (cols k*288) top=sa, tile 8+k bottom=sb
                wgh[g * 128:g * 128 + 64,
                    k * 288:(k + 1) * 288] = w1s[sa].reshape(64, 288)
                wgh[g * 128 + 64:(g + 1) * 128,
                    k * 288:(k + 1) * 288] = w1s[sb].reshape(64, 288)
                # conv1 bias: bank a (cols 2g) = samples sa at 32*k..; bank b = sb
                b1h[32 * k:32 * (k + 1), 2 * g] = b1s[sa]
                b1h[32 * k:32 * (k + 1), 2 * g + 1] = b1s[sb]
                # conv2 weights: tile (32k, 64h): h=0 -> sa, h=1 -> sb
                wgh[g * 128 + 32 * k:g * 128 + 32 * (k + 1),
                    1152:1728] = w2sb[sa].reshape(32, 576)
                wgh[g * 128 + 32 * k:g * 128 + 32 * (k + 1),
                    1728:2304] = w2sb[sb].reshape(32, 576)
        in_maps.append(dict(xqg=xqg, wg=wgh, b1g=b1h, b2zg=b2zh))
    return in_maps


def kernel(x, y_index, y_hard, z, W1, b1, W2, b2, _trace=False):
    x = np.asarray(x, dtype=np.float32)
    z = np.asarray(z, dtype=np.float32)
    W1 = np.asarray(W1, dtype=np.float32)
    b1 = np.asarray(b1, dtype=np.float32)
    W2 = np.asarray(W2, dtype=np.float32)
    b2 = np.asarray(b2, dtype=np.float32)

    nc = _get_nc()
    in_maps = _host_prep(x, y_index, z, W1, b1, W2, b2)
    res = run_bass_kernel_spmd(nc, in_maps, core_ids=list(range(M)), trace=_trace)
    out = np.empty((B, C, HW, HW), np.float32)
    for cr in range(M):
        o = np.asarray(res.results[cr]["out"], dtype=np.float32)
        o = o.reshape(NG, 4, 2, C, HW, WP)[..., 0:HW]  # strip junk cols
        for g in range(NG):
            for k in range(4):
                out[cr * BS + 8 * g + k] = o[g, k, 0]
                out[cr * BS + 8 * g + 4 + k] = o[g, k, 1]
    if _trace:
        kernel._last_results = res
    return out


# revision 5
# speedup vs baseline: 1.1182x; 1.0339x over previous
"""Trainium2 Bass kernel for nn_DecSwitchedDeconv — PE-array-tiled per-sample convs.

Strategy (data-parallel, 32 samples/core, groups of 8):
  - conv1 runs as 8 concurrent (64x32) PE tiles (2 row-groups x 4 col-groups):
    tile k = sample k of the group, 9 taps x 3 chunks of N=362 accumulate into
    2 PSUM banks ([128,362] = 4 samples' 32-cout slices each).
  - conv2 runs as 8 concurrent (32x64) PE tiles: bank r holds samples (r, r+4)
    as 64-cout halves.
  - All routing/gather work happens on host: per-sample weight stacks are
    gathered by y_index, kernels flipped/transposed, z folded into W2 and
    b2*z folded into the residual input xq = x + b2*z. On-chip epilogue is a
    single tensor_tensor add (psum + xq). relu(x) = relu(xq - b2z) via
    tensor_scalar on gpsimd; conv1 bias+relu via scalar ACT from PSUM.
  - I/O in bf16 (x-residual and output upcast on host), span layout (34-wide
    padded rows) so every op is a contiguous [128, N] slab.
"""

import numpy as np

import concourse.bacc as bacc
import concourse.bass as bass
import concourse.mybir as mybir
import concourse.tile as tile
from concourse.bass_utils import run_bass_kernel_spmd

B, C, CSM, NB, HW = 256, 64, 32, 8, 32
M = 8                    # cores
BS = B // M              # 32 samples/core
NG = BS // 8             # 4 groups of 8 samples
WP = HW + 2              # 34
L = WP * WP              # 1156
NVAL = (HW - 1) * WP + HW  # 1086 span covering all valid outputs
BASE = WP + 1            # 35
NCH = 3
CH = NVAL // NCH         # 362
NBUF = 2

f32 = mybir.dt.float32
bf16 = mybir.dt.bfloat16


def _build_bass():
    nc = bacc.Bacc(target_bir_lowering=False, debug=False)
    # per-core inputs (host pre-gathered/packed, all static)
    xqg = nc.dram_tensor("xqg", [NG * 128, 4 * NVAL], bf16, kind="ExternalInput")
    wg = nc.dram_tensor("wg", [NG * 128, 2304], bf16, kind="ExternalInput")
    b1g = nc.dram_tensor("b1g", [128, 2 * NG], f32, kind="ExternalInput")
    b2zg = nc.dram_tensor("b2zg", [128, 4 * NG], f32, kind="ExternalInput")
    outd = nc.dram_tensor("out", [NG * 4 * 128, HW * WP], bf16, kind="ExternalOutput")

    add = mybir.AluOpType.add
    Relu = mybir.ActivationFunctionType.Relu

    with tile.TileContext(nc) as tc:
        # persistent padded tensors; borders zeroed once and never rewritten
        xpads, hpas, hpbs = [], [], []
        for i in range(NBUF):
            xp = nc.alloc_sbuf_tensor(f"xpad{i}", [128, 4 * L], bf16).ap()
            ha = nc.alloc_sbuf_tensor(f"hpa{i}", [128, L], bf16).ap()
            hb = nc.alloc_sbuf_tensor(f"hpb{i}", [128, L], bf16).ap()
            for k in range(4):
                nc.vector.memset(xp[:, k * L:k * L + BASE], 0.0)
                nc.vector.memset(xp[:, k * L + BASE + NVAL:(k + 1) * L], 0.0)
            nc.gpsimd.memset(ha[:, 0:BASE], 0.0)
            nc.gpsimd.memset(ha[:, BASE + NVAL:L], 0.0)
            nc.gpsimd.memset(hb[:, 0:BASE], 0.0)
            nc.gpsimd.memset(hb[:, BASE + NVAL:L], 0.0)
            xpads.append(xp)
            hpas.append(ha)
            hpbs.append(hb)
        warm_sb = nc.alloc_sbuf_tensor("warm_sb", [128, 640], bf16).ap()
        b1t = nc.alloc_sbuf_tensor("b1t", [128, 2 * NG], f32).ap()
        b2zt = nc.alloc_sbuf_tensor("b2zt", [128, 4 * NG], f32).ap()
        nc.sync.dma_start(b2zt, b2zg.ap())

        with (
            tc.tile_pool(name="io", bufs=3) as iop,
            tc.tile_pool(name="wp", bufs=3) as wpp,
            tc.tile_pool(name="osp", bufs=3) as ospp,
            tc.tile_pool(name="ps", bufs=8, space="PSUM") as psp,
        ):
            # HAM pre-warm: dummy full-array matmuls while first loads land
            wps = psp.tile([128, CH], f32, name="warm_ps", tag="ps")
            for i in range(28):
                nc.tensor.matmul(
                    wps[:, 0:256], lhsT=warm_sb[:, 512:640],
                    rhs=warm_sb[:, 0:256], start=True, stop=True)

            def emit_loads(g):
                xqall = iop.tile([128, 4 * NVAL], bf16, tag="xq", name=f"xq_{g}")
                wsb = wpp.tile([128, 2304], bf16, tag="w", name=f"w_{g}")
                nc.gpsimd.dma_start(wsb[:, :], wg.ap()[g * 128:(g + 1) * 128, :])
                if g == 0:
                    for k in range(4):
                        nc.sync.dma_start(
                            xqall[:, k * NVAL:(k + 1) * NVAL],
                            xqg.ap()[g * 128:(g + 1) * 128, k * NVAL:(k + 1) * NVAL])
                    nc.sync.dma_start(b1t, b1g.ap())
                else:
                    nc.sync.dma_start(xqall[:, 0:2 * NVAL],
                                      xqg.ap()[g * 128:(g + 1) * 128, 0:2 * NVAL])
                    nc.sync.dma_start(xqall[:, 2 * NVAL:4 * NVAL],
                                      xqg.ap()[g * 128:(g + 1) * 128, 2 * NVAL:4 * NVAL])
                return xqall, wsb

            def emit_relu(g, xqs, xpad):
                xp3 = xpad.rearrange("p (b h w) -> p b h w", b=4, w=WP)
                for k in range(4):
                    if g == 0:
                        nc.vector.tensor_scalar(
                            xpad[:, k * L + BASE:k * L + BASE + NVAL], xqs[k][:, :],
                            b2zt[:, 4 * g + k:4 * g + k + 1], 0.0,
                            op0=add, op1=mybir.AluOpType.max)
                    else:
                        nc.scalar.activation(
                            xpad[:, k * L + BASE:k * L + BASE + NVAL], xqs[k][:, :],
                            Relu, bias=b2zt[:, 4 * g + k:4 * g + k + 1])
                    nc.gpsimd.memset(xp3[:, k, 1:HW + 1, 0:WP:WP - 1], 0.0)

            def emit_conv1(g, w1sb, xpad, ha, hb):
                for c in range(NCH):
                    psa = psp.tile([128, CH], f32, name=f"ps1a_{g}_{c}", tag="ps")
                    psb = psp.tile([128, CH], f32, name=f"ps1b_{g}_{c}", tag="ps")
                    for t in range(9):
                        dy, dx = divmod(t, 3)
                        off = dy * WP + dx + c * CH
                        for k in range(8):
                            row = 64 * (k // 4)
                            col = 32 * (k % 4)
                            ps = psa if k < 4 else psb
                            nc.tensor.matmul(
                                ps[col:col + 32, :],
                                lhsT=w1sb[row:row + 64,
                                          (k % 4) * 288 + t * 32:(k % 4) * 288 + (t + 1) * 32],
                                rhs=xpad[row:row + 64,
                                         (k % 4) * L + off:(k % 4) * L + off + CH],
                                start=(t == 0), stop=(t == 8),
                                tile_position=(row, col),
                            )
                    nc.vector.tensor_scalar(
                        ha[:, BASE + c * CH:BASE + (c + 1) * CH], psa[:, :],
                        b1t[:, 2 * g:2 * g + 1], 0.0,
                        op0=add, op1=mybir.AluOpType.max)
                    nc.vector.tensor_scalar(
                        hb[:, BASE + c * CH:BASE + (c + 1) * CH], psb[:, :],
                        b1t[:, 2 * g + 1:2 * g + 2], 0.0,
                        op0=add, op1=mybir.AluOpType.max)
                ha3 = ha.rearrange("p (h w) -> p h w", w=WP)
                hb3 = hb.rearrange("p (h w) -> p h w", w=WP)
                nc.gpsimd.memset(ha3[:, 1:HW + 1, 0:WP:WP - 1], 0.0)
                nc.gpsimd.memset(hb3[:, 1:HW + 1, 0:WP:WP - 1], 0.0)

            def emit_conv2(g, w2sb, xqs, ha, hb):
                outs = []
                for r in range(4):
                    osp = ospp.tile([128, 32 * WP], bf16, tag=f"os{r}",
                                    name=f"os{r}_{g}")
                    outs.append(osp)
                for c in range(NCH):
                    pss = [psp.tile([128, CH], f32, name=f"ps2_{g}_{c}_{r}", tag="ps")
                           for r in range(4)]
                    for t in range(9):
                        dy, dx = divmod(t, 3)
                        off = dy * WP + dx + c * CH
                        for k in range(8):
                            r, h = divmod(k, 2)
                            src = ha if h == 0 else hb
                            nc.tensor.matmul(
                                pss[r][64 * h:64 * h + 64, :],
                                lhsT=w2sb[32 * r:32 * r + 32,
                                          h * 576 + t * 64:h * 576 + (t + 1) * 64],
                                rhs=src[32 * r:32 * r + 32, off:off + CH],
                                start=(t == 0), stop=(t == 8),
                                tile_position=(32 * r, 64 * h),
                            )
                    for r in range(4):
                        nc.vector.tensor_tensor(
                            outs[r][:, c * CH:(c + 1) * CH], pss[r][:, :],
                            xqs[r][:, c * CH:(c + 1) * CH], op=add)
                        eng = nc.sync if r % 2 == 0 else nc.gpsimd
                        eng.dma_start(
                            outd.ap()[(g * 4 + r) * 128:(g * 4 + r + 1) * 128,
                                      c * CH:(c + 1) * CH],
                            outs[r][:, c * CH:(c + 1) * CH])

            # pair-structured emission: conv1(2p), conv1(2p+1), conv2(2p),
            # conv2(2p+1) — halves PE tiling-mode switches and overlaps the
            # second group's relu/evac under same-mode matmul streams.
            pending = emit_loads(0)
            for p in range(NG // 2):
                ga, gb = 2 * p, 2 * p + 1
                sa, sb = [], []
                for g, st in ((ga, sa), (gb, sb)):
                    xqall, wsb = pending if g == ga else pending_b
                    st.extend([
                        xpads[g % NBUF], hpas[g % NBUF], hpbs[g % NBUF],
                        [xqall[:, k * NVAL:(k + 1) * NVAL] for k in range(4)],
                        wsb[:, 0:1152], wsb[:, 1152:2304],
                    ])
                    if g == ga:
                        emit_relu(ga, sa[3], sa[0])
                        pending_b = emit_loads(gb)
                xpad_a, ha_a, hb_a, xqs_a, w1_a, w2_a = sa
                xpad_b, ha_b, hb_b, xqs_b, w1_b, w2_b = sb
                emit_conv1(ga, w1_a, xpad_a, ha_a, hb_a)
                emit_relu(gb, xqs_b, xpad_b)
                emit_conv1(gb, w1_b, xpad_b, ha_b, hb_b)
                if gb + 1 < NG:
                    pending = emit_loads(gb + 1)
                emit_conv2(ga, w2_a, xqs_a, ha_a, hb_a)
                emit_conv2(gb, w2_b, xqs_b, ha_b, hb_b)


    nc.compile()
    return nc


import os as _os
if _os.environ.get("LDWOPT", "0") == "1":
    import concourse.bass_utils as _bu
    if not getattr(_bu, "_ldw_patched", False):
        _orig = _bu.run_command
        def _rc(argv, **kw):
            argv = ["--enable-ldw-opt=true" if a == "--enable-ldw-opt=false" else a
                    for a in argv]
            return _orig(argv, **kw)
        _bu.run_command = _rc
        _bu._ldw_patched = True

_NC = None


def _get_nc():
    global _NC
    if _NC is None:
        _NC = _build_bass()
    return _NC


def _host_prep(x, y_index, z, W1, b1, W2, b2):
    import ml_dtypes
    idx = np.asarray(y_index).reshape(B).astype(np.int64)
    # flipped-kernel stacks: w1t [NB, C, 9, CSM], w2t [NB, CSM, 9, C]
    w1t = np.ascontiguousarray(
        W1[:, :, :, ::-1, ::-1].transpose(0, 1, 3, 4, 2)).reshape(NB, C, 9, CSM)
    w2t = np.ascontiguousarray(
        W2[:, :, :, ::-1, ::-1].transpose(0, 1, 3, 4, 2)).reshape(NB, CSM, 9, C)
    w1s = w1t[idx]                                   # [B, 64, 9, 32] f32
    w2s = w2t[idx] * z[:, None, None, :]             # [B, 32, 9, 64] f32
    b2z = b2[idx] * z                                # [B, 64]
    b1s = b1[idx]                                    # [B, 32]

    # xq = x + b2z, span layout [B, 64, 1086] with zeros at junk cols
    xq = x + b2z[:, :, None, None]
    xqp = np.zeros((B, C, WP, WP), np.float32)
    xqp[:, :, 1:HW + 1, 1:HW + 1] = xq
    xq_span = xqp.reshape(B, C, L)[:, :, BASE:BASE + NVAL].copy()
    ji = np.array([i for i in range(NVAL) if (BASE + i) % WP in (0, WP - 1)])
    xq_span[:, :, ji] = -1e30
    xq_span = xq_span.astype(ml_dtypes.bfloat16)

    w1sb = w1s.astype(ml_dtypes.bfloat16)
    w2sb = w2s.astype(ml_dtypes.bfloat16)

    in_maps = []
    for cr in range(M):
        s0 = cr * BS
        # xqg rows: (g, k) pair tile = samples (s0+8g+k | s0+8g+4+k)
        xqg = np.empty((NG * 128, 4 * NVAL), ml_dtypes.bfloat16)
        wgh = np.zeros((NG * 128, 2304), ml_dtypes.bfloat16)
        b1h = np.zeros((128, 2 * NG), np.float32)
        b2zh = np.zeros((128, 4 * NG), np.float32)
        for g in range(NG):
            for k in range(4):
                sa, sb = s0 + 8 * g + k, s0 + 8 * g + 4 + k
                r0 = g * 128
                xqg[r0:r0 + 64, k * NVAL:(k + 1) * NVAL] = xq_span[sa]
                xqg[r0 + 64:r0 + 128, k * NVAL:(k + 1) * NVAL] = xq_span[sb]
                b2zh[0:64, 4 * g + k] = -b2z[sa]
                b2zh[64:128, 4 * g + k] = b2z[sb]
                # conv1 weights: tile k (cols k*288) top=sa, tile 8+k bottom=sb
                wgh[g * 128:g * 128 + 64,
                    k * 288:(k + 1) * 288] = w1s[sa].reshape(64, 288)
                wgh[g * 128 + 64:(g + 1) * 128,
                    k * 288:(k + 1) * 288] = w1s[sb].reshape(64, 288)
                # conv1 bias: bank a (cols 2g) = samples sa at 32*k..; bank b = sb
                b1h[32 * k:32 * (k + 1), 2 * g] = b1s[sa]
                b1h[32 * k:32 * (k + 1), 2 * g + 1] = b1s[sb]
                # conv2 weights: tile (32k, 64h): h=0 -> sa, h=1 -> sb
                wgh[g * 128 + 32 * k:g * 128 + 32 * (k + 1),
                    1152:1728] = w2sb[sa].reshape(32, 576)
                wgh[g * 128 + 32 * k:g * 128 + 32 * (k + 1),
                    1728:2304] = w2sb[sb].reshape(32, 576)
        in_maps.append(dict(xqg=xqg, wg=wgh, b1g=b1h, b2zg=b2zh))
    return in_maps


def kernel(x, y_index, y_hard, z, W1, b1, W2, b2, _trace=False):
    x = np.asarray(x, dtype=np.float32)
    z = np.asarray(z, dtype=np.float32)
    W1 = np.asarray(W1, dtype=np.float32)
    b1 = np.asarray(b1, dtype=np.float32)
    W2 = np.asarray(W2, dtype=np.float32)
    b2 = np.asarray(b2, dtype=np.float32)

    nc = _get_nc()
    in_maps = _host_prep(x, y_index, z, W1, b1, W2, b2)
    res = run_bass_kernel_spmd(nc, in_maps, core_ids=list(range(M)), trace=_trace)
    out = np.empty((B, C, HW, HW), np.float32)
    for cr in range(M):
        o = np.asarray(res.results[cr]["out"], dtype=np.float32)
        o = o.reshape(NG, 4, 2, C, HW, WP)[..., 0:HW]  # strip junk cols
        for g in range(NG):
            for k in range(4):
                out[cr * BS + 8 * g + k] = o[g, k, 0]
                out[cr * BS + 8 * g + 4 + k] = o[g, k, 1]
    if _trace:
        kernel._last_results = res
    return out
